# revision 36
# baseline (speedup 1.0000x reference)
"""Trainium2 Bass kernel for the YOLO-style DetectionLayer loss.

Data parallel over batch: 4 samples/core on 8 cores.  The six losses
depend on x only at the 80 ground-truth cells per core, plus a closed-form
constant for the empty-cell class loss.  x is fed channel-last and
anchor-interleaved (c-major: channel block c holds the 3 anchors
adjacently), padded to 256 channels per cell, so each GT cell's channels
are one aligned 1KB run in DRAM and the 3 label logits of a class are
contiguous.

Per core:
  1. sync issues the single small input DMA (cy) first; the act-table
     load runs on scalar in parallel; iota-derived masks (eye, lower
     -triangle) are built before cy even lands.
  2. vector computes the cell indices (floor via round(x-0.5), exploiting
     the HW round-to-nearest cast) and gpsimd issues two indirect DMAs:
     the 128 cells' channel rows, and the 3 contiguous label logits.
  3. during the gather flight: PE transposes the cell key for duplicate
     detection, ln(sa), ln(gw) and the tw/th loss offsets are prepared.
  4. after the gather: one Exp activation + k15-add + fast-reciprocal
     gives sigmoid(x,y,conf) and exp(w,h) per anchor; a short vector
     chain computes the center-form IoU and best anchor w3; one PE
     matmul (cellmask @ w3) + a fused STT resolves last-write-wins
     dedup; per-anchor lse comes from scalar-engine Exp-accumulate.
  5. candidate columns are reduced with w3; losses are squared diffs;
     one final PE matmul with weights -(live) reduces the 80 rows.
Host combines the 8x8 output scalars (negate, x25 conf, empty-cell class
constant) -- the data-parallel psum of the sharding hint.
"""

import numpy as np

import concourse.bacc as bacc
import concourse.bass as bass
import concourse.mybir as mybir
import concourse.tile as tile
from concourse.bass import IndirectOffsetOnAxis
from concourse.bass_utils import run_bass_kernel_spmd

# Problem shape (hardcoded per harness contract).
BS, GS, NA, NCLS, NGT = 32, 52, 3, 80, 20
NCORES = 8
BPC = BS // NCORES          # samples per core
P = 128
NGTC = BPC * NGT            # ground truths per core (80)
CH = 5 + NCLS               # channels per anchor (85)
NCH = NA * CH               # 255 real channels
CHP = 256                   # padded channels per cell (1KB aligned)
PLANE = GS * GS             # 2704 cells per sample
XF2 = BPC * PLANE * CHP
CYW = 32                    # cy columns

F32 = mybir.dt.float32
BF16 = mybir.dt.bfloat16
I32 = mybir.dt.int32
A = mybir.AluOpType
AF = mybir.ActivationFunctionType
AX = mybir.AxisListType


def _patch_act_tables():
    """Steer the act-table chooser so Exp and Ln both resolve to the one
    set that contains both (`natural_log_exp_and_others`) -> exactly one
    ACT_TABLE_LOAD in the kernel."""
    from concourse import hw_specs, bacc as bacc_mod
    orig = hw_specs.get_activation_tables

    def patched(arch):
        t = {k: set(v) for k, v in orig(arch).items()}
        if "natural_log_exp_and_others" in t:
            for name in t:
                if name != "natural_log_exp_and_others":
                    t[name] = t[name] - {AF.Exp, AF.Ln}
        return t

    hw_specs.get_activation_tables = patched
    bacc_mod.get_activation_tables = patched
    return orig


def _unpatch_act_tables(orig):
    from concourse import hw_specs, bacc as bacc_mod
    hw_specs.get_activation_tables = orig
    bacc_mod.get_activation_tables = orig


def _build():
    nc = bacc.Bacc("TRN2", target_bir_lowering=False, debug=False,
                   num_devices=NCORES)
    xf = nc.dram_tensor("xf", [XF2, 1], F32, kind="ExternalInput")
    cy_d = nc.dram_tensor("cy", [P, CYW], F32, kind="ExternalInput")
    out_d = nc.dram_tensor("out", [1, 8], F32, kind="ExternalOutput")

    v, s, g, te, sy = nc.vector, nc.scalar, nc.gpsimd, nc.tensor, nc.sync

    with tile.TileContext(nc) as tc:
        with tc.tile_pool(name="sb", bufs=1) as sb, \
             tc.tile_pool(name="ps", bufs=1, space="PSUM") as ps:
            cy = sb.tile([P, CYW], F32)
            ytx = cy[:, 0:2]           # xc, yc
            lblr = cy[:, 4:5]          # class label (float, exact int)
            bpl256 = cy[:, 5:6]        # b_local * PLANE * 256
            ancr = cy[:, 8:14]         # anchors c-major (w0 w1 w2 h0 h1 h2)
            k15 = cy[:, 16:31]         # 1x6, 0x6, 1x3 (c-major)

            gt = sb.tile([P, CHP], F32)   # c-major cell channels

            # ---------------- scalar: the one input DMA ----------------
            # (scalar's preamble finishes ~0.7us before sync's; with a
            #  single act-table load the HWDGE ring is FIFO: cy completes
            #  before the table-load transfer)
            s.dma_start(out=cy[:], in_=cy_d[:])

            # ---------------- pre-cy device constants ------------------
            pid = sb.tile([P, 1], F32)
            g.iota(out=pid[:], pattern=[[0, 1]], base=0, channel_multiplier=1,
                   allow_small_or_imprecise_dtypes=True)
            io128 = sb.tile([P, P], F32)
            g.iota(out=io128[:], pattern=[[1, P]], base=0,
                   channel_multiplier=0, allow_small_or_imprecise_dtypes=True)
            validt = sb.tile([P, 1], F32)      # 1.0 for p < 80
            v.tensor_scalar(out=validt[:], in0=pid[:], scalar1=float(NGTC),
                            scalar2=None, op0=A.is_lt)
            eye = sb.tile([P, P], F32)
            v.tensor_scalar(out=eye[:], in0=io128[:], scalar1=pid[:, 0:1],
                            scalar2=None, op0=A.is_equal)
            lowm = sb.tile([P, P], F32)        # (col < p) & (p < 80)
            v.tensor_scalar(out=lowm[:], in0=io128[:], scalar1=pid[:, 0:1],
                            scalar2=validt[:, 0:1], op0=A.is_lt, op1=A.mult)

            # ---------------- vector: index chain ----------------------
            # floor via round-to-nearest of (x - 0.5): the HW cast rounds
            # to nearest and 52*uniform is never exactly k or k+1/2.
            gxy01h = sb.tile([P, 2], F32)
            v.tensor_scalar(out=gxy01h[:], in0=ytx, scalar1=float(GS),
                            scalar2=-0.5, op0=A.mult, op1=A.add)
            ci = sb.tile([P, 2], I32)
            v.tensor_copy(out=ci[:], in_=gxy01h[:])
            ijf = sb.tile([P, 2], F32)         # floor(gx), floor(gy)
            v.tensor_copy(out=ijf[:], in_=ci[:])
            tcell = sb.tile([P, 1], F32)       # gj*52*256 + b*PLANE*256
            v.tensor_scalar(out=tcell[:], in0=ijf[:, 1:2],
                            scalar1=float(GS * CHP), scalar2=bpl256,
                            op0=A.mult, op1=A.add)
            idx1f = sb.tile([P, 1], F32)       # + gi*256  (the cell key)
            v.tensor_scalar(out=idx1f[:], in0=ijf[:, 0:1],
                            scalar1=float(CHP), scalar2=tcell[:, 0:1],
                            op0=A.mult, op1=A.add)
            idx1 = sb.tile([P, 1], I32)
            v.tensor_copy(out=idx1[:], in_=idx1f[:])
            # label-logit offset: idx1 + 15 + 3*lbl (3 contiguous floats)
            u3 = sb.tile([P, 1], F32)          # 3*lbl + 15 (early)
            v.tensor_scalar(out=u3[:], in0=lblr, scalar1=3.0, scalar2=15.0,
                            op0=A.mult, op1=A.add)
            idx2f = sb.tile([P, 1], F32)
            v.tensor_scalar(out=idx2f[:], in0=u3[:], scalar1=idx1f[:, 0:1],
                            scalar2=None, op0=A.add)
            idx2 = sb.tile([P, 1], I32)
            v.tensor_copy(out=idx2[:], in_=idx2f[:])

            # ---------------- gpsimd: constants + the gathers ----------
            sa = sb.tile([P, 6], F32)          # anchors / stride, c-major
            g.tensor_scalar_mul(out=sa[:], in0=ancr, scalar1=1.0 / (416 // GS))
            gwh = sb.tile([P, 2], F32)         # gw, gh (grid units)
            g.tensor_scalar_mul(out=gwh[:], in0=cy[:, 2:4], scalar1=float(GS))
            gwhh = sb.tile([P, 2], F32)        # gw/2, gh/2
            g.tensor_scalar_mul(out=gwhh[:], in0=gwh[:], scalar1=0.5)
            areag = sb.tile([P, 1], F32)       # gw*gh
            g.tensor_tensor(out=areag[:], in0=gwh[:, 0:1], in1=gwh[:, 1:2],
                            op=A.mult)
            gxy01 = sb.tile([P, 2], F32)
            g.tensor_scalar_mul(out=gxy01[:], in0=ytx, scalar1=float(GS))

            # gather 1: 128 descriptors, one 1KB cell row each
            g.indirect_dma_start(
                out=gt[0:P, 0:CHP], out_offset=None, in_=xf[:],
                in_offset=IndirectOffsetOnAxis(ap=idx1[0:P, 0:1], axis=0))
            # gather 2: the 3 label logits (contiguous in c-major layout)
            lg = sb.tile([P, 3], F32)
            g.indirect_dma_start(
                out=lg[0:P, 0:3], out_offset=None, in_=xf[:],
                in_offset=IndirectOffsetOnAxis(ap=idx2[0:P, 0:1], axis=0))

            # ---- gpsimd work during the gather flight ----
            loss = sb.tile([P, 8], BF16)       # bf16: single-pass final MM
            g.memset(loss[:, 6:7], 1.0)        # live-count column
            g.memset(loss[:, 7:8], 0.0)
            tgt = sb.tile([P, 2], F32)         # frac targets (x, y)
            g.tensor_tensor(out=tgt[:, 0:2], in0=gxy01[:], in1=ijf[:],
                            op=A.subtract)

            # ---------------- scalar during the flight ------------------
            lnsa = sb.tile([P, 6], F32)        # ln(sa), c-major
            s.activation(out=lnsa[:], in_=sa[:], func=AF.Ln, scale=1.0)
            lngwh = sb.tile([P, 2], F32)       # ln(gw), ln(gh)
            s.activation(out=lngwh[:], in_=gwh[:], func=AF.Ln, scale=1.0)

            pre6 = sb.tile([P, 6], F32)        # ln(sa) - ln(gw), c-major
            g.tensor_tensor(
                out=pre6[:, 0:6].rearrange("p (c a) -> p c a", c=2),
                in0=lnsa[:, 0:6].rearrange("p (c a) -> p c a", c=2),
                in1=lngwh[:, 0:2].rearrange("p (c o) -> p c o", c=2)
                    .to_broadcast((P, 2, NA)),
                op=A.subtract)

            # ---------------- PE: cell-key transpose during flight ------
            tpq = ps.tile([P, P], F32)
            te.transpose(out=tpq[:], in_=idx1f[:, 0:1].to_broadcast((P, P)),
                         identity=eye[:])
            m1 = sb.tile([P, P], F32)          # key[p] == key[col]
            v.tensor_scalar(out=m1[:], in0=tpq[:], scalar1=idx1f[:, 0:1],
                            scalar2=None, op0=A.is_equal)
            cm = sb.tile([P, P], BF16)         # ... & (col < p) & valid_p
            v.tensor_tensor(out=cm[:], in0=m1[:], in1=lowm[:], op=A.mult)

            # ---------------- post-gather: activations -------------------
            ex15 = sb.tile([P, 15], F32)       # exp(-v) of channels 0..4
            s.activation(out=ex15[:], in_=gt[:, 0:15], func=AF.Exp, scale=-1.0)

            # vector: sigmoid/exp via the k15 trick (c-major throughout)
            a15 = sb.tile([P, 15], F32)
            v.tensor_tensor(out=a15[:], in0=ex15[:], in1=k15, op=A.add)
            r15 = sb.tile([P, 15], F32)        # sigx3, sigy3, e^tw3, e^th3, sigc3
            v.reciprocal_approx_fast(out=r15[:], in_=a15[:])
            dx6 = sb.tile([P, 6], F32)
            v.tensor_tensor(out=dx6[:, 0:6].rearrange("p (c a) -> p c a", c=2),
                            in0=r15[:, 0:6].rearrange("p (c a) -> p c a", c=2),
                            in1=tgt[:, 0:2].rearrange("p (c o) -> p c o", c=2)
                                .to_broadcast((P, 2, NA)),
                            op=A.subtract)

            # per-anchor sum(exp(cls)) via act accumulate (scalar engine)
            clsv = gt[:, 15:NCH].rearrange("p (j a) -> p a j", a=NA)
            ez = sb.tile([P, NA * NCLS], F32)
            s3 = sb.tile([P, 3], F32)
            for a in range(NA):
                s.activation(out=ez[:, a * NCLS:(a + 1) * NCLS],
                             in_=clsv[:, a, :], func=AF.Exp, scale=1.0,
                             accum_out=s3[:, a:a + 1])
            lse3 = sb.tile([P, 3], F32)
            s.activation(out=lse3[:], in_=s3[:], func=AF.Ln, scale=1.0)

            # ---- gpsimd post-gather (parallel with vector IoU chain) ----
            # candidates carry the target pre-subtracted: sum_a w3 = 1, so
            # sel - tgt == sum_a w3*(cand_a - tgt); the post-select subtract
            # disappears.
            cand = sb.tile([P, 18], F32)       # c-major [c=6][a=3]
            g.tensor_tensor(out=cand[:, 6:12], in0=gt[:, 6:12], in1=pre6[:],
                            op=A.add)
            g.tensor_tensor(out=cand[:, 0:6].rearrange("p (c a) -> p c a", c=2),
                            in0=r15[:, 0:6].rearrange("p (c a) -> p c a", c=2),
                            in1=tgt[:, 0:2].rearrange("p (c o) -> p c o", c=2)
                                .to_broadcast((P, 2, NA)),
                            op=A.subtract)
            # c5 = lse - label logit (gathered)
            g.tensor_tensor(out=cand[:, 15:18], in0=lse3[:], in1=lg[:],
                            op=A.subtract)

            # ---------------- vector: IoU chain -> w3 --------------------
            bwh6 = sb.tile([P, 6], F32)        # bw3, bh3 (c-major)
            v.tensor_tensor(out=bwh6[:], in0=r15[:, 6:12], in1=sa[:],
                            op=A.mult)
            ext0 = sb.tile([P, 6], F32)        # bw/2 + gw/2
            v.scalar_tensor_tensor(
                out=ext0[:, 0:6].rearrange("p (c a) -> p c a", c=2),
                in0=bwh6[:, 0:6].rearrange("p (c a) -> p c a", c=2),
                scalar=0.5,
                in1=gwhh[:, 0:2].rearrange("p (c o) -> p c o", c=2)
                    .to_broadcast((P, 2, NA)),
                op0=A.mult, op1=A.add)
            emin = sb.tile([P, 6], F32)        # ext0 - dx
            v.tensor_sub(emin[:], ext0[:], dx6[:])
            epl = sb.tile([P, 6], F32)         # ext0 + dx (on gpsimd)
            g.tensor_add(epl[:], ext0[:], dx6[:])
            ovl = sb.tile([P, 6], F32)         # ext0 - |dx|
            v.tensor_tensor(out=ovl[:], in0=emin[:], in1=epl[:], op=A.min)
            v.tensor_tensor(out=ovl[:], in0=ovl[:], in1=bwh6[:], op=A.min)
            v.tensor_tensor(out=ovl[:, 0:6].rearrange("p (c a) -> p c a", c=2),
                            in0=ovl[:, 0:6].rearrange("p (c a) -> p c a", c=2),
                            in1=gwh[:, 0:2].rearrange("p (c o) -> p c o", c=2)
                                .to_broadcast((P, 2, NA)),
                            op=A.min)
            # overlap > 0 guaranteed: gw,gh >= 2.6 grid units, |dx| <= 1
            inter = sb.tile([P, 3], F32)
            v.tensor_tensor(out=inter[:], in0=ovl[:, 0:3], in1=ovl[:, 3:6],
                            op=A.mult)
            areab = sb.tile([P, 3], F32)
            g.tensor_tensor(out=areab[:], in0=bwh6[:, 0:3], in1=bwh6[:, 3:6],
                            op=A.mult)
            union = sb.tile([P, 3], F32)       # areab + areag - inter
            v.scalar_tensor_tensor(out=union[:], in0=areab[:],
                                   scalar=areag[:, 0:1], in1=inter[:],
                                   op0=A.add, op1=A.subtract)
            rcu = sb.tile([P, 3], F32)
            v.reciprocal_approx_fast(out=rcu[:], in_=union[:])
            iou = sb.tile([P, 3], F32)
            v.tensor_mul(iou[:], inter[:], rcu[:])
            mx = sb.tile([P, 1], F32)          # best iou (the conf target)
            v.tensor_reduce(out=mx[:], in_=iou[:], axis=AX.X, op=A.max)
            w3 = sb.tile([P, 3], BF16)         # best-anchor one-hot (exact)
            v.tensor_scalar(out=w3[:], in0=iou[:], scalar1=mx[:, 0:1],
                            scalar2=None, op0=A.is_equal)
            g.tensor_tensor(out=cand[:, 12:15], in0=r15[:, 12:15],
                            in1=mx[:, 0:1].to_broadcast((P, 3)),
                            op=A.subtract)

            # ---------------- dedup: matmul + fused STT ------------------
            psK = ps.tile([P, 3], F32)
            te.matmul(out=psK[0:P, 0:3], lhsT=cm[:], rhs=w3[:],
                      start=True, stop=True)
            junk3 = sb.tile([P, 3], F32)
            t1 = sb.tile([P, 1], F32)          # dup count for this row
            v.scalar_tensor_tensor(out=junk3[:], in0=psK[:], scalar=1.0,
                                   in1=w3[:], op0=A.mult, op1=A.mult,
                                   accum_out=t1[:])
            neglive = sb.tile([P, 1], BF16)    # min(t,1) - valid = -(live)
            v.tensor_scalar(out=neglive[:], in0=t1[:], scalar1=1.0,
                            scalar2=validt[:, 0:1], op0=A.min,
                            op1=A.subtract)

            # ---------------- select + losses ----------------------------
            prod18 = sb.tile([P, 18], F32)
            v.tensor_tensor(out=prod18[:, 0:18].rearrange("p (c a) -> p c a", c=6),
                            in0=cand[:, 0:18].rearrange("p (c a) -> p c a", c=6),
                            in1=w3[:, 0:3].rearrange("p (o a) -> p o a", o=1)
                                .to_broadcast((P, 6, NA)),
                            op=A.mult)
            with nc.allow_low_precision("bf16 loss columns, ~0.4% << 2e-2 gate"):
                v.tensor_reduce(out=loss[:, 0:6],
                                in_=prod18[:, 0:18]
                                    .rearrange("p (c a) -> p c a", c=6),
                                axis=AX.X, op=A.add)
            v.tensor_tensor(out=loss[:, 0:5], in0=loss[:, 0:5],
                            in1=loss[:, 0:5], op=A.mult)

            # ---------------- final row reduction + output ---------------
            psL = ps.tile([P, 8], F32)
            te.matmul(out=psL[0:1, 0:8], lhsT=neglive[:, 0:1],
                      rhs=loss[:, 0:8], start=True, stop=True)
            outs = sb.tile([P, 8], F32)
            v.tensor_copy(out=outs[0:1, :], in_=psL[0:1, :])
            sy.dma_start(out=out_d[:], in_=outs[0:1, 0:8])

    orig = _patch_act_tables()
    try:
        nc.compile()
    finally:
        _unpatch_act_tables(orig)
    return nc


_CACHE = {}


def _get_nc():
    if "nc" not in _CACHE:
        _CACHE["nc"] = _build()
    return _CACHE["nc"]


def _make_cy(y_true_shard, anchors):
    """Per-core y_true + per-partition index constants: [P, CYW]."""
    cy = np.zeros((P, CYW), np.float32)
    b_local = np.repeat(np.arange(BPC), NGT).astype(np.float32)
    cy[:, 0:4] = [0.5, 0.5, 0.25, 0.25]     # pad rows: benign box
    cy[:NGTC, 0:5] = y_true_shard.reshape(NGTC, 5)
    cy[:NGTC, 5] = b_local * PLANE * CHP
    cy[:, 8:14] = anchors.T.reshape(1, 6)    # c-major anchors
    cy[:, 16:31] = np.repeat([1.0, 1.0, 0.0, 0.0, 1.0], NA)[None, :]
    return np.ascontiguousarray(cy)


def make_in_maps(x, y_true, anchors):
    x = np.asarray(x, np.float32)
    y_true = np.ascontiguousarray(y_true, np.float32)
    anchors = np.asarray(anchors, np.float32)
    # channel-last, anchor-interleaved (c-major), padded to 256 channels:
    # [b, gj, gi, c*3+a] so one cell's channels are one aligned 1KB run
    # and a class's 3 anchor logits are contiguous (layout permutation
    # only, applied while sharding).
    xt = np.zeros((BS, GS, GS, CHP), np.float32)
    xt[..., :NCH] = (x.reshape(BS, NA, CH, GS, GS)
                     .transpose(0, 3, 4, 2, 1).reshape(BS, GS, GS, NCH))
    in_maps = []
    for c in range(NCORES):
        in_maps.append({
            "xf": xt[c * BPC:(c + 1) * BPC].reshape(XF2, 1),
            "cy": _make_cy(y_true[c * BPC:(c + 1) * BPC], anchors),
        })
    return in_maps


def combine_outputs(results):
    cols = np.stack([np.asarray(r["out"], np.float64)[0] for r in results])
    tot = -cols.sum(axis=0)      # device sums are weighted by -(live)
    n_live = tot[6]
    out = np.empty(6, np.float64)
    out[0:4] = tot[0:4]
    out[4] = tot[5] + (BS * NA * PLANE - n_live) * np.log(np.float64(NCLS))
    out[5] = tot[4] * 25.0
    return out.astype(np.float32)


def run(x, y_true, anchors, trace=False, **kwargs):
    nc = _get_nc()
    res = run_bass_kernel_spmd(nc, make_in_maps(x, y_true, anchors),
                               list(range(NCORES)), trace=trace, **kwargs)
    return combine_outputs(res.results), res


def kernel(x, y_true, anchors):
    out, _ = run(x, y_true, anchors)
    return out


# revision 39
# speedup vs baseline: 1.1009x; 1.1009x over previous
"""Trainium2 Bass kernel for the YOLO-style DetectionLayer loss.

Data parallel over batch: 4 samples/core on 8 cores.  The six losses
depend on x only at the 80 ground-truth cells per core, plus a closed-form
constant for the empty-cell class loss.  x is fed channel-last and
anchor-interleaved (c-major: channel block c holds the 3 anchors
adjacently), padded to 256 channels per cell, so each GT cell's channels
are one aligned 1KB run in DRAM and the 3 label logits of a class are
contiguous.

Per core:
  1. sync issues the single small input DMA (cy) first; the act-table
     load runs on scalar in parallel; iota-derived masks (eye, lower
     -triangle) are built before cy even lands.
  2. vector computes the cell indices (floor via round(x-0.5), exploiting
     the HW round-to-nearest cast) and gpsimd issues two indirect DMAs:
     the 128 cells' channel rows, and the 3 contiguous label logits.
  3. during the gather flight: PE transposes the cell key for duplicate
     detection, ln(sa), ln(gw) and the tw/th loss offsets are prepared.
  4. after the gather: one Exp activation + k15-add + fast-reciprocal
     gives sigmoid(x,y,conf) and exp(w,h) per anchor; a short vector
     chain computes the center-form IoU and best anchor w3; one PE
     matmul (cellmask @ w3) + a fused STT resolves last-write-wins
     dedup; per-anchor lse comes from scalar-engine Exp-accumulate.
  5. candidate columns are reduced with w3; losses are squared diffs;
     one final PE matmul with weights -(live) reduces the 80 rows.
Host combines the 8x8 output scalars (negate, x25 conf, empty-cell class
constant) -- the data-parallel psum of the sharding hint.
"""

import numpy as np

import concourse.bacc as bacc
import concourse.bass as bass
import concourse.mybir as mybir
import concourse.tile as tile
from concourse.bass import IndirectOffsetOnAxis
from concourse.bass_utils import run_bass_kernel_spmd

# Problem shape (hardcoded per harness contract).
BS, GS, NA, NCLS, NGT = 32, 52, 3, 80, 20
NCORES = 8
BPC = BS // NCORES          # samples per core
P = 128
NGTC = BPC * NGT            # ground truths per core (80)
CH = 5 + NCLS               # channels per anchor (85)
NCH = NA * CH               # 255 real channels
CHP = 256                   # padded channels per cell (1KB aligned)
PLANE = GS * GS             # 2704 cells per sample
XF2 = BPC * PLANE * CHP
CYW = 32                    # cy columns

F32 = mybir.dt.float32
BF16 = mybir.dt.bfloat16
I32 = mybir.dt.int32
A = mybir.AluOpType
AF = mybir.ActivationFunctionType
AX = mybir.AxisListType


def _patch_act_tables():
    """Steer the act-table chooser so Exp and Ln both resolve to the one
    set that contains both (`natural_log_exp_and_others`) -> exactly one
    ACT_TABLE_LOAD in the kernel."""
    from concourse import hw_specs, bacc as bacc_mod
    orig = hw_specs.get_activation_tables

    def patched(arch):
        t = {k: set(v) for k, v in orig(arch).items()}
        if "natural_log_exp_and_others" in t:
            for name in t:
                if name != "natural_log_exp_and_others":
                    t[name] = t[name] - {AF.Exp, AF.Ln}
        return t

    hw_specs.get_activation_tables = patched
    bacc_mod.get_activation_tables = patched
    return orig


def _unpatch_act_tables(orig):
    from concourse import hw_specs, bacc as bacc_mod
    hw_specs.get_activation_tables = orig
    bacc_mod.get_activation_tables = orig


def _build():
    nc = bacc.Bacc("TRN2", target_bir_lowering=False, debug=False,
                   num_devices=NCORES)
    xf = nc.dram_tensor("xf", [XF2, 1], F32, kind="ExternalInput")
    cy_d = nc.dram_tensor("cy", [P, CYW], F32, kind="ExternalInput")
    out_d = nc.dram_tensor("out", [1, 8], F32, kind="ExternalOutput")

    v, s, g, te, sy = nc.vector, nc.scalar, nc.gpsimd, nc.tensor, nc.sync

    with tile.TileContext(nc) as tc:
        with tc.tile_pool(name="sb", bufs=1) as sb, \
             tc.tile_pool(name="ps", bufs=1, space="PSUM") as ps:
            cy = sb.tile([P, CYW], F32)
            ytx = cy[:, 0:2]           # xc, yc
            lblr = cy[:, 4:5]          # class label (float, exact int)
            bpl256 = cy[:, 5:6]        # b_local * PLANE * 256
            ancr = cy[:, 8:14]         # anchors c-major (w0 w1 w2 h0 h1 h2)
            k15 = cy[:, 16:31]         # 1x6, 0x6, 1x3 (c-major)

            gt = sb.tile([P, CHP], F32)   # c-major cell channels

            # ---------------- sync: the one input DMA ------------------
            # (sync: scalar's ring would queue cy's completion behind the
            #  act-table load, measured +2us; vector can't issue DMAs)
            sy.dma_start(out=cy[:], in_=cy_d[:])

            # ---------------- pre-cy device constants ------------------
            pid = sb.tile([P, 1], F32)
            g.iota(out=pid[:], pattern=[[0, 1]], base=0, channel_multiplier=1,
                   allow_small_or_imprecise_dtypes=True)
            io128 = sb.tile([P, P], F32)
            g.iota(out=io128[:], pattern=[[1, P]], base=0,
                   channel_multiplier=0, allow_small_or_imprecise_dtypes=True)
            validt = sb.tile([P, 1], F32)      # 1.0 for p < 80
            v.tensor_scalar(out=validt[:], in0=pid[:], scalar1=float(NGTC),
                            scalar2=None, op0=A.is_lt)
            eye = sb.tile([P, P], F32)
            v.tensor_scalar(out=eye[:], in0=io128[:], scalar1=pid[:, 0:1],
                            scalar2=None, op0=A.is_equal)
            lowm = sb.tile([P, P], F32)        # (col < p) & (p < 80)
            v.tensor_scalar(out=lowm[:], in0=io128[:], scalar1=pid[:, 0:1],
                            scalar2=validt[:, 0:1], op0=A.is_lt, op1=A.mult)

            # ---------------- vector: index chain ----------------------
            # floor via round-to-nearest of (x - 0.5): the HW cast rounds
            # to nearest and 52*uniform is never exactly k or k+1/2.
            gxy01h = sb.tile([P, 2], F32)
            v.tensor_scalar(out=gxy01h[:], in0=ytx, scalar1=float(GS),
                            scalar2=-0.5, op0=A.mult, op1=A.add)
            ci = sb.tile([P, 2], I32)
            v.tensor_copy(out=ci[:], in_=gxy01h[:])
            ijf = sb.tile([P, 2], F32)         # floor(gx), floor(gy)
            v.tensor_copy(out=ijf[:], in_=ci[:])
            tcell = sb.tile([P, 1], F32)       # gj*52*256 + b*PLANE*256
            v.tensor_scalar(out=tcell[:], in0=ijf[:, 1:2],
                            scalar1=float(GS * CHP), scalar2=bpl256,
                            op0=A.mult, op1=A.add)
            idx1f = sb.tile([P, 1], F32)       # + gi*256  (the cell key)
            v.tensor_scalar(out=idx1f[:], in0=ijf[:, 0:1],
                            scalar1=float(CHP), scalar2=tcell[:, 0:1],
                            op0=A.mult, op1=A.add)
            idx1 = sb.tile([P, 1], I32)
            v.tensor_copy(out=idx1[:], in_=idx1f[:])
            # label-logit offset: idx1 + 15 + 3*lbl (3 contiguous floats)
            u3 = sb.tile([P, 1], F32)          # 3*lbl + 15 (early)
            v.tensor_scalar(out=u3[:], in0=lblr, scalar1=3.0, scalar2=15.0,
                            op0=A.mult, op1=A.add)
            idx2f = sb.tile([P, 1], F32)
            v.tensor_scalar(out=idx2f[:], in0=u3[:], scalar1=idx1f[:, 0:1],
                            scalar2=None, op0=A.add)
            idx2 = sb.tile([P, 1], I32)
            v.tensor_copy(out=idx2[:], in_=idx2f[:])

            # ---------------- gpsimd: constants + the gathers ----------
            sa = sb.tile([P, 6], F32)          # anchors / stride, c-major
            g.tensor_scalar_mul(out=sa[:], in0=ancr, scalar1=1.0 / (416 // GS))
            gwh = sb.tile([P, 2], F32)         # gw, gh (grid units)
            g.tensor_scalar_mul(out=gwh[:], in0=cy[:, 2:4], scalar1=float(GS))
            gwhh = sb.tile([P, 2], F32)        # gw/2, gh/2
            g.tensor_scalar_mul(out=gwhh[:], in0=gwh[:], scalar1=0.5)
            areag = sb.tile([P, 1], F32)       # gw*gh
            g.tensor_tensor(out=areag[:], in0=gwh[:, 0:1], in1=gwh[:, 1:2],
                            op=A.mult)
            gxy01 = sb.tile([P, 2], F32)
            g.tensor_scalar_mul(out=gxy01[:], in0=ytx, scalar1=float(GS))

            # gather 1: 128 descriptors, one 1KB cell row each
            g.indirect_dma_start(
                out=gt[0:P, 0:CHP], out_offset=None, in_=xf[:],
                in_offset=IndirectOffsetOnAxis(ap=idx1[0:P, 0:1], axis=0))
            # gather 2: the 3 label logits (contiguous in c-major layout)
            lg = sb.tile([P, 3], F32)
            g.indirect_dma_start(
                out=lg[0:P, 0:3], out_offset=None, in_=xf[:],
                in_offset=IndirectOffsetOnAxis(ap=idx2[0:P, 0:1], axis=0))

            # ---- gpsimd work during the gather flight ----
            loss = sb.tile([P, 8], BF16)       # bf16: single-pass final MM
            g.memset(loss[:, 6:7], 1.0)        # live-count column
            g.memset(loss[:, 7:8], 0.0)
            tgt = sb.tile([P, 2], F32)         # frac targets (x, y)
            g.tensor_tensor(out=tgt[:, 0:2], in0=gxy01[:], in1=ijf[:],
                            op=A.subtract)

            # ---------------- scalar during the flight ------------------
            lnsa = sb.tile([P, 6], F32)        # ln(sa), c-major
            s.activation(out=lnsa[:], in_=sa[:], func=AF.Ln, scale=1.0)
            lngwh = sb.tile([P, 2], F32)       # ln(gw), ln(gh)
            s.activation(out=lngwh[:], in_=gwh[:], func=AF.Ln, scale=1.0)

            pre6 = sb.tile([P, 6], F32)        # ln(sa) - ln(gw), c-major
            g.tensor_tensor(
                out=pre6[:, 0:6].rearrange("p (c a) -> p c a", c=2),
                in0=lnsa[:, 0:6].rearrange("p (c a) -> p c a", c=2),
                in1=lngwh[:, 0:2].rearrange("p (c o) -> p c o", c=2)
                    .to_broadcast((P, 2, NA)),
                op=A.subtract)

            # ---------------- PE: cell-key transpose during flight ------
            tpq = ps.tile([P, P], F32)
            te.transpose(out=tpq[:], in_=idx1f[:, 0:1].to_broadcast((P, P)),
                         identity=eye[:])
            m1 = sb.tile([P, P], F32)          # key[p] == key[col]
            v.tensor_scalar(out=m1[:], in0=tpq[:], scalar1=idx1f[:, 0:1],
                            scalar2=None, op0=A.is_equal)
            cm = sb.tile([P, P], BF16)         # ... & (col < p) & valid_p
            v.tensor_tensor(out=cm[:], in0=m1[:], in1=lowm[:], op=A.mult)

            # ---------------- post-gather: activations -------------------
            ex15 = sb.tile([P, 15], F32)       # exp(-v) of channels 0..4
            s.activation(out=ex15[:], in_=gt[:, 0:15], func=AF.Exp, scale=-1.0)

            # vector: sigmoid/exp via the k15 trick (c-major throughout)
            a15 = sb.tile([P, 15], F32)
            v.tensor_tensor(out=a15[:], in0=ex15[:], in1=k15, op=A.add)
            r15 = sb.tile([P, 15], F32)        # sigx3, sigy3, e^tw3, e^th3, sigc3
            v.reciprocal_approx_fast(out=r15[:], in_=a15[:])
            dx6 = sb.tile([P, 6], F32)
            v.tensor_tensor(out=dx6[:, 0:6].rearrange("p (c a) -> p c a", c=2),
                            in0=r15[:, 0:6].rearrange("p (c a) -> p c a", c=2),
                            in1=tgt[:, 0:2].rearrange("p (c o) -> p c o", c=2)
                                .to_broadcast((P, 2, NA)),
                            op=A.subtract)

            # per-anchor sum(exp(cls)) via act accumulate (scalar engine)
            clsv = gt[:, 15:NCH].rearrange("p (j a) -> p a j", a=NA)
            ez = sb.tile([P, NA * NCLS], F32)
            s3 = sb.tile([P, 3], F32)
            for a in range(NA):
                s.activation(out=ez[:, a * NCLS:(a + 1) * NCLS],
                             in_=clsv[:, a, :], func=AF.Exp, scale=1.0,
                             accum_out=s3[:, a:a + 1])
            lse3 = sb.tile([P, 3], F32)
            s.activation(out=lse3[:], in_=s3[:], func=AF.Ln, scale=1.0)

            # ---- gpsimd post-gather (parallel with vector IoU chain) ----
            # candidates carry the target pre-subtracted: sum_a w3 = 1, so
            # sel - tgt == sum_a w3*(cand_a - tgt); the post-select subtract
            # disappears.
            cand = sb.tile([P, 18], F32)       # c-major [c=6][a=3]
            g.tensor_tensor(out=cand[:, 6:12], in0=gt[:, 6:12], in1=pre6[:],
                            op=A.add)
            g.tensor_tensor(out=cand[:, 0:6].rearrange("p (c a) -> p c a", c=2),
                            in0=r15[:, 0:6].rearrange("p (c a) -> p c a", c=2),
                            in1=tgt[:, 0:2].rearrange("p (c o) -> p c o", c=2)
                                .to_broadcast((P, 2, NA)),
                            op=A.subtract)
            # c5 = lse - label logit (gathered)
            g.tensor_tensor(out=cand[:, 15:18], in0=lse3[:], in1=lg[:],
                            op=A.subtract)

            # ---------------- vector: IoU chain -> w3 --------------------
            bwh6 = sb.tile([P, 6], F32)        # bw3, bh3 (c-major)
            v.tensor_tensor(out=bwh6[:], in0=r15[:, 6:12], in1=sa[:],
                            op=A.mult)
            ext0 = sb.tile([P, 6], F32)        # bw/2 + gw/2
            v.scalar_tensor_tensor(
                out=ext0[:, 0:6].rearrange("p (c a) -> p c a", c=2),
                in0=bwh6[:, 0:6].rearrange("p (c a) -> p c a", c=2),
                scalar=0.5,
                in1=gwhh[:, 0:2].rearrange("p (c o) -> p c o", c=2)
                    .to_broadcast((P, 2, NA)),
                op0=A.mult, op1=A.add)
            emin = sb.tile([P, 6], F32)        # ext0 - dx
            v.tensor_sub(emin[:], ext0[:], dx6[:])
            epl = sb.tile([P, 6], F32)         # ext0 + dx (on gpsimd)
            g.tensor_add(epl[:], ext0[:], dx6[:])
            ovl = sb.tile([P, 6], F32)         # ext0 - |dx|
            v.tensor_tensor(out=ovl[:], in0=emin[:], in1=epl[:], op=A.min)
            v.tensor_tensor(out=ovl[:], in0=ovl[:], in1=bwh6[:], op=A.min)
            v.tensor_tensor(out=ovl[:, 0:6].rearrange("p (c a) -> p c a", c=2),
                            in0=ovl[:, 0:6].rearrange("p (c a) -> p c a", c=2),
                            in1=gwh[:, 0:2].rearrange("p (c o) -> p c o", c=2)
                                .to_broadcast((P, 2, NA)),
                            op=A.min)
            # overlap > 0 guaranteed: gw,gh >= 2.6 grid units, |dx| <= 1
            inter = sb.tile([P, 3], F32)
            v.tensor_tensor(out=inter[:], in0=ovl[:, 0:3], in1=ovl[:, 3:6],
                            op=A.mult)
            areab = sb.tile([P, 3], F32)
            g.tensor_tensor(out=areab[:], in0=bwh6[:, 0:3], in1=bwh6[:, 3:6],
                            op=A.mult)
            union = sb.tile([P, 3], F32)       # areab + areag - inter
            v.scalar_tensor_tensor(out=union[:], in0=areab[:],
                                   scalar=areag[:, 0:1], in1=inter[:],
                                   op0=A.add, op1=A.subtract)
            rcu = sb.tile([P, 3], F32)
            v.reciprocal_approx_fast(out=rcu[:], in_=union[:])
            iou = sb.tile([P, 3], F32)
            v.tensor_mul(iou[:], inter[:], rcu[:])
            mx = sb.tile([P, 1], F32)          # best iou (the conf target)
            v.tensor_reduce(out=mx[:], in_=iou[:], axis=AX.X, op=A.max)
            w3 = sb.tile([P, 3], BF16)         # best-anchor one-hot (exact)
            v.tensor_scalar(out=w3[:], in0=iou[:], scalar1=mx[:, 0:1],
                            scalar2=None, op0=A.is_equal)
            g.tensor_tensor(out=cand[:, 12:15], in0=r15[:, 12:15],
                            in1=mx[:, 0:1].to_broadcast((P, 3)),
                            op=A.subtract)

            # ---------------- dedup: matmul + fused STT ------------------
            psK = ps.tile([P, 3], F32)
            te.matmul(out=psK[0:P, 0:3], lhsT=cm[:], rhs=w3[:],
                      start=True, stop=True)
            junk3 = sb.tile([P, 3], F32)
            t1 = sb.tile([P, 1], F32)          # dup count for this row
            v.scalar_tensor_tensor(out=junk3[:], in0=psK[:], scalar=1.0,
                                   in1=w3[:], op0=A.mult, op1=A.mult,
                                   accum_out=t1[:])
            neglive = sb.tile([P, 1], BF16)    # min(t,1) - valid = -(live)
            v.tensor_scalar(out=neglive[:], in0=t1[:], scalar1=1.0,
                            scalar2=validt[:, 0:1], op0=A.min,
                            op1=A.subtract)

            # ---------------- select + losses ----------------------------
            prod18 = sb.tile([P, 18], F32)
            v.tensor_tensor(out=prod18[:, 0:18].rearrange("p (c a) -> p c a", c=6),
                            in0=cand[:, 0:18].rearrange("p (c a) -> p c a", c=6),
                            in1=w3[:, 0:3].rearrange("p (o a) -> p o a", o=1)
                                .to_broadcast((P, 6, NA)),
                            op=A.mult)
            with nc.allow_low_precision("bf16 loss columns, ~0.4% << 2e-2 gate"):
                v.tensor_reduce(out=loss[:, 0:6],
                                in_=prod18[:, 0:18]
                                    .rearrange("p (c a) -> p c a", c=6),
                                axis=AX.X, op=A.add)
            v.tensor_tensor(out=loss[:, 0:5], in0=loss[:, 0:5],
                            in1=loss[:, 0:5], op=A.mult)

            # ---------------- final row reduction + output ---------------
            psL = ps.tile([P, 8], F32)
            te.matmul(out=psL[0:1, 0:8], lhsT=neglive[:, 0:1],
                      rhs=loss[:, 0:8], start=True, stop=True)
            outs = sb.tile([P, 8], F32)
            v.tensor_copy(out=outs[0:1, :], in_=psL[0:1, :])
            sy.dma_start(out=out_d[:], in_=outs[0:1, 0:8])

    orig = _patch_act_tables()
    try:
        nc.compile()
    finally:
        _unpatch_act_tables(orig)
    return nc


_CACHE = {}


def _get_nc():
    if "nc" not in _CACHE:
        _CACHE["nc"] = _build()
    return _CACHE["nc"]


def _make_cy(y_true_shard, anchors):
    """Per-core y_true + per-partition index constants: [P, CYW]."""
    cy = np.zeros((P, CYW), np.float32)
    b_local = np.repeat(np.arange(BPC), NGT).astype(np.float32)
    cy[:, 0:4] = [0.5, 0.5, 0.25, 0.25]     # pad rows: benign box
    cy[:NGTC, 0:5] = y_true_shard.reshape(NGTC, 5)
    cy[:NGTC, 5] = b_local * PLANE * CHP
    cy[:, 8:14] = anchors.T.reshape(1, 6)    # c-major anchors
    cy[:, 16:31] = np.repeat([1.0, 1.0, 0.0, 0.0, 1.0], NA)[None, :]
    return np.ascontiguousarray(cy)


def make_in_maps(x, y_true, anchors):
    x = np.asarray(x, np.float32)
    y_true = np.ascontiguousarray(y_true, np.float32)
    anchors = np.asarray(anchors, np.float32)
    # channel-last, anchor-interleaved (c-major), padded to 256 channels:
    # [b, gj, gi, c*3+a] so one cell's channels are one aligned 1KB run
    # and a class's 3 anchor logits are contiguous (layout permutation
    # only, applied while sharding).
    xt = np.zeros((BS, GS, GS, CHP), np.float32)
    xt[..., :NCH] = (x.reshape(BS, NA, CH, GS, GS)
                     .transpose(0, 3, 4, 2, 1).reshape(BS, GS, GS, NCH))
    in_maps = []
    for c in range(NCORES):
        in_maps.append({
            "xf": xt[c * BPC:(c + 1) * BPC].reshape(XF2, 1),
            "cy": _make_cy(y_true[c * BPC:(c + 1) * BPC], anchors),
        })
    return in_maps


def combine_outputs(results):
    cols = np.stack([np.asarray(r["out"], np.float64)[0] for r in results])
    tot = -cols.sum(axis=0)      # device sums are weighted by -(live)
    n_live = tot[6]
    out = np.empty(6, np.float64)
    out[0:4] = tot[0:4]
    out[4] = tot[5] + (BS * NA * PLANE - n_live) * np.log(np.float64(NCLS))
    out[5] = tot[4] * 25.0
    return out.astype(np.float32)


def run(x, y_true, anchors, trace=False, **kwargs):
    nc = _get_nc()
    res = run_bass_kernel_spmd(nc, make_in_maps(x, y_true, anchors),
                               list(range(NCORES)), trace=trace, **kwargs)
    return combine_outputs(res.results), res


def kernel(x, y_true, anchors):
    out, _ = run(x, y_true, anchors)
    return out


# revision 42
# speedup vs baseline: 1.1081x; 1.0065x over previous
"""Trainium2 Bass kernel for the YOLO-style DetectionLayer loss.

Data parallel over batch: 4 samples/core on 8 cores.  The six losses
depend on x only at the 80 ground-truth cells per core, plus a closed-form
constant for the empty-cell class loss.  x is fed channel-last and
anchor-interleaved (c-major: channel block c holds the 3 anchors
adjacently), padded to 256 channels per cell, so each GT cell's channels
are one aligned 1KB run in DRAM and the 3 label logits of a class are
contiguous.

Per core:
  1. sync issues the single small input DMA (cy) first; the act-table
     load runs on scalar in parallel; iota-derived masks (eye, lower
     -triangle) are built before cy even lands.
  2. vector computes the cell indices (floor via round(x-0.5), exploiting
     the HW round-to-nearest cast) and gpsimd issues two indirect DMAs:
     the 128 cells' channel rows, and the 3 contiguous label logits.
  3. during the gather flight: PE transposes the cell key for duplicate
     detection, ln(sa), ln(gw) and the tw/th loss offsets are prepared.
  4. after the gather: one Exp activation + k15-add + fast-reciprocal
     gives sigmoid(x,y,conf) and exp(w,h) per anchor; a short vector
     chain computes the center-form IoU and best anchor w3; one PE
     matmul (cellmask @ w3) + a fused STT resolves last-write-wins
     dedup; per-anchor lse comes from scalar-engine Exp-accumulate.
  5. candidate columns are reduced with w3; losses are squared diffs;
     one final PE matmul with weights -(live) reduces the 80 rows.
Host combines the 8x8 output scalars (negate, x25 conf, empty-cell class
constant) -- the data-parallel psum of the sharding hint.
"""

import numpy as np

import concourse.bacc as bacc
import concourse.bass as bass
import concourse.mybir as mybir
import concourse.tile as tile
from concourse.bass import IndirectOffsetOnAxis
from concourse.bass_utils import run_bass_kernel_spmd

# Problem shape (hardcoded per harness contract).
BS, GS, NA, NCLS, NGT = 32, 52, 3, 80, 20
NCORES = 8
BPC = BS // NCORES          # samples per core
P = 128
NGTC = BPC * NGT            # ground truths per core (80)
CH = 5 + NCLS               # channels per anchor (85)
NCH = NA * CH               # 255 real channels
CHP = 256                   # padded channels per cell (1KB aligned)
PLANE = GS * GS             # 2704 cells per sample
XF2 = BPC * PLANE * CHP
CYW = 32                    # cy columns

F32 = mybir.dt.float32
BF16 = mybir.dt.bfloat16
I32 = mybir.dt.int32
A = mybir.AluOpType
AF = mybir.ActivationFunctionType
AX = mybir.AxisListType


def _patch_act_tables():
    """Steer the act-table chooser so Exp and Ln both resolve to the one
    set that contains both (`natural_log_exp_and_others`) -> exactly one
    ACT_TABLE_LOAD in the kernel."""
    from concourse import hw_specs, bacc as bacc_mod
    orig = hw_specs.get_activation_tables

    def patched(arch):
        t = {k: set(v) for k, v in orig(arch).items()}
        if "natural_log_exp_and_others" in t:
            for name in t:
                if name != "natural_log_exp_and_others":
                    t[name] = t[name] - {AF.Exp, AF.Ln}
        return t

    hw_specs.get_activation_tables = patched
    bacc_mod.get_activation_tables = patched
    return orig


def _unpatch_act_tables(orig):
    from concourse import hw_specs, bacc as bacc_mod
    hw_specs.get_activation_tables = orig
    bacc_mod.get_activation_tables = orig


def _build():
    nc = bacc.Bacc("TRN2", target_bir_lowering=False, debug=False,
                   num_devices=NCORES)
    xf = nc.dram_tensor("xf", [XF2, 1], F32, kind="ExternalInput")
    cy_d = nc.dram_tensor("cy", [P, CYW], F32, kind="ExternalInput")
    out_d = nc.dram_tensor("out", [1, 8], F32, kind="ExternalOutput")

    v, s, g, te, sy = nc.vector, nc.scalar, nc.gpsimd, nc.tensor, nc.sync

    with tile.TileContext(nc) as tc:
        with tc.tile_pool(name="sb", bufs=1) as sb, \
             tc.tile_pool(name="ps", bufs=1, space="PSUM") as ps:
            cy = sb.tile([P, CYW], F32)
            ytx = cy[:, 0:2]           # xc, yc
            lblr = cy[:, 4:5]          # class label (float, exact int)
            bpl256 = cy[:, 5:6]        # b_local * PLANE * 256
            ancr = cy[:, 8:14]         # anchors c-major (w0 w1 w2 h0 h1 h2)
            k15 = cy[:, 16:31]         # 1x6, 0x6, 1x3 (c-major)

            gt = sb.tile([P, CHP], F32)   # c-major cell channels

            # ---------------- sync: the one input DMA ------------------
            # (sync: scalar's ring would queue cy's completion behind the
            #  act-table load, measured +2us; vector can't issue DMAs)
            sy.dma_start(out=cy[:], in_=cy_d[:])

            # ---------------- pre-cy device constants ------------------
            pid = sb.tile([P, 1], F32)
            g.iota(out=pid[:], pattern=[[0, 1]], base=0, channel_multiplier=1,
                   allow_small_or_imprecise_dtypes=True)
            io128 = sb.tile([P, P], F32)
            g.iota(out=io128[:], pattern=[[1, P]], base=0,
                   channel_multiplier=0, allow_small_or_imprecise_dtypes=True)
            validt = sb.tile([P, 1], F32)      # 1.0 for p < 80
            v.tensor_scalar(out=validt[:], in0=pid[:], scalar1=float(NGTC),
                            scalar2=None, op0=A.is_lt)
            eye = sb.tile([P, P], F32)
            v.tensor_scalar(out=eye[:], in0=io128[:], scalar1=pid[:, 0:1],
                            scalar2=None, op0=A.is_equal)
            lowm = sb.tile([P, P], F32)        # (col < p) & (p < 80)
            v.tensor_scalar(out=lowm[:], in0=io128[:], scalar1=pid[:, 0:1],
                            scalar2=validt[:, 0:1], op0=A.is_lt, op1=A.mult)

            # ---------------- vector: index chain ----------------------
            # floor via round-to-nearest of (x - 0.5): the HW cast rounds
            # to nearest and 52*uniform is never exactly k or k+1/2.
            gxy01h = sb.tile([P, 2], F32)
            v.tensor_scalar(out=gxy01h[:], in0=ytx, scalar1=float(GS),
                            scalar2=-0.5, op0=A.mult, op1=A.add)
            ci = sb.tile([P, 2], I32)
            v.tensor_copy(out=ci[:], in_=gxy01h[:])
            ijf = sb.tile([P, 2], F32)         # floor(gx), floor(gy)
            v.tensor_copy(out=ijf[:], in_=ci[:])
            tcell = sb.tile([P, 1], F32)       # gj*52*256 + b*PLANE*256
            v.tensor_scalar(out=tcell[:], in0=ijf[:, 1:2],
                            scalar1=float(GS * CHP), scalar2=bpl256,
                            op0=A.mult, op1=A.add)
            idx1f = sb.tile([P, 1], F32)       # + gi*256  (the cell key)
            v.tensor_scalar(out=idx1f[:], in0=ijf[:, 0:1],
                            scalar1=float(CHP), scalar2=tcell[:, 0:1],
                            op0=A.mult, op1=A.add)
            idx1 = sb.tile([P, 1], I32)
            v.tensor_copy(out=idx1[:], in_=idx1f[:])
            # label-logit offset: idx1 + 15 + 3*lbl (3 contiguous floats)
            u3 = sb.tile([P, 1], F32)          # 3*lbl + 15 (early)
            v.tensor_scalar(out=u3[:], in0=lblr, scalar1=3.0, scalar2=15.0,
                            op0=A.mult, op1=A.add)
            idx2f = sb.tile([P, 1], F32)
            v.tensor_scalar(out=idx2f[:], in0=u3[:], scalar1=idx1f[:, 0:1],
                            scalar2=None, op0=A.add)
            idx2 = sb.tile([P, 1], I32)
            v.tensor_copy(out=idx2[:], in_=idx2f[:])

            # ---------------- gpsimd: constants + the gathers ----------
            sa = sb.tile([P, 6], F32)          # anchors / stride, c-major
            g.tensor_scalar_mul(out=sa[:], in0=ancr, scalar1=1.0 / (416 // GS))
            gwh = sb.tile([P, 2], F32)         # gw, gh (grid units)
            g.tensor_scalar_mul(out=gwh[:], in0=cy[:, 2:4], scalar1=float(GS))
            gwhh = sb.tile([P, 2], F32)        # gw/2, gh/2
            g.tensor_scalar_mul(out=gwhh[:], in0=gwh[:], scalar1=0.5)
            areag = sb.tile([P, 1], F32)       # gw*gh
            g.tensor_tensor(out=areag[:], in0=gwh[:, 0:1], in1=gwh[:, 1:2],
                            op=A.mult)
            gxy01 = sb.tile([P, 2], F32)
            g.tensor_scalar_mul(out=gxy01[:], in0=ytx, scalar1=float(GS))

            # gather 1: 128 descriptors, one 1KB cell row each
            g.indirect_dma_start(
                out=gt[0:P, 0:CHP], out_offset=None, in_=xf[:],
                in_offset=IndirectOffsetOnAxis(ap=idx1[0:P, 0:1], axis=0))
            # gather 2: the 3 label logits (contiguous in c-major layout)
            lg = sb.tile([P, 3], F32)
            g.indirect_dma_start(
                out=lg[0:P, 0:3], out_offset=None, in_=xf[:],
                in_offset=IndirectOffsetOnAxis(ap=idx2[0:P, 0:1], axis=0))

            # ---- gpsimd work during the gather flight ----
            loss = sb.tile([P, 8], BF16)       # bf16: single-pass final MM
            g.memset(loss[:, 6:7], 1.0)        # live-count column
            g.memset(loss[:, 7:8], 0.0)
            tgt = sb.tile([P, 2], F32)         # frac targets (x, y)
            g.tensor_tensor(out=tgt[:, 0:2], in0=gxy01[:], in1=ijf[:],
                            op=A.subtract)

            # ---------------- scalar during the flight ------------------
            lnsa = sb.tile([P, 6], F32)        # ln(sa), c-major
            s.activation(out=lnsa[:], in_=sa[:], func=AF.Ln, scale=1.0)
            lngwh = sb.tile([P, 2], F32)       # ln(gw), ln(gh)
            s.activation(out=lngwh[:], in_=gwh[:], func=AF.Ln, scale=1.0)

            pre6 = sb.tile([P, 6], F32)        # ln(sa) - ln(gw), c-major
            g.tensor_tensor(
                out=pre6[:, 0:6].rearrange("p (c a) -> p c a", c=2),
                in0=lnsa[:, 0:6].rearrange("p (c a) -> p c a", c=2),
                in1=lngwh[:, 0:2].rearrange("p (c o) -> p c o", c=2)
                    .to_broadcast((P, 2, NA)),
                op=A.subtract)

            # ---------------- PE: cell-key transpose during flight ------
            tpq = ps.tile([P, P], F32)
            te.transpose(out=tpq[:], in_=idx1f[:, 0:1].to_broadcast((P, P)),
                         identity=eye[:])
            m1 = sb.tile([P, P], F32)          # key[p] == key[col]
            v.tensor_scalar(out=m1[:], in0=tpq[:], scalar1=idx1f[:, 0:1],
                            scalar2=None, op0=A.is_equal)
            cm = sb.tile([P, P], BF16)         # ... & (col < p) & valid_p
            v.tensor_tensor(out=cm[:], in0=m1[:], in1=lowm[:], op=A.mult)

            # ---------------- post-gather: activations -------------------
            ex15 = sb.tile([P, 15], F32)       # exp(-v) of channels 0..4
            s.activation(out=ex15[:], in_=gt[:, 0:15], func=AF.Exp, scale=-1.0)

            # vector: sigmoid/exp via the k15 trick (c-major throughout)
            a15 = sb.tile([P, 15], F32)
            v.tensor_tensor(out=a15[:], in0=ex15[:], in1=k15, op=A.add)
            r15 = sb.tile([P, 15], F32)        # sigx3, sigy3, e^tw3, e^th3, sigc3
            v.reciprocal_approx_fast(out=r15[:], in_=a15[:])
            dx6 = sb.tile([P, 6], F32)
            v.tensor_tensor(out=dx6[:, 0:6].rearrange("p (c a) -> p c a", c=2),
                            in0=r15[:, 0:6].rearrange("p (c a) -> p c a", c=2),
                            in1=tgt[:, 0:2].rearrange("p (c o) -> p c o", c=2)
                                .to_broadcast((P, 2, NA)),
                            op=A.subtract)

            # per-anchor sum(exp(cls)) via act accumulate (scalar engine)
            clsv = gt[:, 15:NCH].rearrange("p (j a) -> p a j", a=NA)
            ez = sb.tile([P, NA * NCLS], F32)
            s3 = sb.tile([P, 3], F32)
            for a in range(NA):
                s.activation(out=ez[:, a * NCLS:(a + 1) * NCLS],
                             in_=clsv[:, a, :], func=AF.Exp, scale=1.0,
                             accum_out=s3[:, a:a + 1])
            lse3 = sb.tile([P, 3], F32)
            s.activation(out=lse3[:], in_=s3[:], func=AF.Ln, scale=1.0)

            # ---- gpsimd post-gather (parallel with vector IoU chain) ----
            # w3 is one-hot, so (sum_a w3*(c_a - t))^2 == sum_a w3*(c_a - t)^2:
            # candidates carry (value - target)^2 directly and the post-select
            # subtract and square both disappear from the vector tail.
            cand = sb.tile([P, 18], F32)       # c-major [c=6][a=3]
            g.tensor_tensor(out=cand[:, 6:12], in0=gt[:, 6:12], in1=pre6[:],
                            op=A.add)
            g.tensor_tensor(out=cand[:, 6:12], in0=cand[:, 6:12],
                            in1=cand[:, 6:12], op=A.mult)
            g.tensor_tensor(out=cand[:, 0:6].rearrange("p (c a) -> p c a", c=2),
                            in0=r15[:, 0:6].rearrange("p (c a) -> p c a", c=2),
                            in1=tgt[:, 0:2].rearrange("p (c o) -> p c o", c=2)
                                .to_broadcast((P, 2, NA)),
                            op=A.subtract)
            g.tensor_tensor(out=cand[:, 0:6], in0=cand[:, 0:6],
                            in1=cand[:, 0:6], op=A.mult)
            # c5 = lse - label logit (gathered); stays linear (not squared)
            g.tensor_tensor(out=cand[:, 15:18], in0=lse3[:], in1=lg[:],
                            op=A.subtract)

            # ---------------- vector: IoU chain -> w3 --------------------
            bwh6 = sb.tile([P, 6], F32)        # bw3, bh3 (c-major)
            v.tensor_tensor(out=bwh6[:], in0=r15[:, 6:12], in1=sa[:],
                            op=A.mult)
            ext0 = sb.tile([P, 6], F32)        # bw/2 + gw/2
            v.scalar_tensor_tensor(
                out=ext0[:, 0:6].rearrange("p (c a) -> p c a", c=2),
                in0=bwh6[:, 0:6].rearrange("p (c a) -> p c a", c=2),
                scalar=0.5,
                in1=gwhh[:, 0:2].rearrange("p (c o) -> p c o", c=2)
                    .to_broadcast((P, 2, NA)),
                op0=A.mult, op1=A.add)
            emin = sb.tile([P, 6], F32)        # ext0 - dx
            v.tensor_sub(emin[:], ext0[:], dx6[:])
            epl = sb.tile([P, 6], F32)         # ext0 + dx (on gpsimd)
            g.tensor_add(epl[:], ext0[:], dx6[:])
            ovl = sb.tile([P, 6], F32)         # ext0 - |dx|
            v.tensor_tensor(out=ovl[:], in0=emin[:], in1=epl[:], op=A.min)
            v.tensor_tensor(out=ovl[:], in0=ovl[:], in1=bwh6[:], op=A.min)
            v.tensor_tensor(out=ovl[:, 0:6].rearrange("p (c a) -> p c a", c=2),
                            in0=ovl[:, 0:6].rearrange("p (c a) -> p c a", c=2),
                            in1=gwh[:, 0:2].rearrange("p (c o) -> p c o", c=2)
                                .to_broadcast((P, 2, NA)),
                            op=A.min)
            # overlap > 0 guaranteed: gw,gh >= 2.6 grid units, |dx| <= 1
            inter = sb.tile([P, 3], F32)
            v.tensor_tensor(out=inter[:], in0=ovl[:, 0:3], in1=ovl[:, 3:6],
                            op=A.mult)
            areab = sb.tile([P, 3], F32)
            g.tensor_tensor(out=areab[:], in0=bwh6[:, 0:3], in1=bwh6[:, 3:6],
                            op=A.mult)
            union = sb.tile([P, 3], F32)       # areab + areag - inter
            v.scalar_tensor_tensor(out=union[:], in0=areab[:],
                                   scalar=areag[:, 0:1], in1=inter[:],
                                   op0=A.add, op1=A.subtract)
            rcu = sb.tile([P, 3], F32)
            v.reciprocal_approx_fast(out=rcu[:], in_=union[:])
            iou = sb.tile([P, 3], F32)
            v.tensor_mul(iou[:], inter[:], rcu[:])
            mx = sb.tile([P, 1], F32)          # best iou (the conf target)
            v.tensor_reduce(out=mx[:], in_=iou[:], axis=AX.X, op=A.max)
            w3 = sb.tile([P, 3], BF16)         # best-anchor one-hot (exact)
            v.tensor_scalar(out=w3[:], in0=iou[:], scalar1=mx[:, 0:1],
                            scalar2=None, op0=A.is_equal)
            # conf: per-anchor (sigc_a - iou_a)^2; selection picks the best
            # anchor, where iou_a == best_iou (one-hot exactness)
            g.tensor_tensor(out=cand[:, 12:15], in0=r15[:, 12:15],
                            in1=iou[:], op=A.subtract)
            g.tensor_tensor(out=cand[:, 12:15], in0=cand[:, 12:15],
                            in1=cand[:, 12:15], op=A.mult)

            # ---------------- dedup: matmul + fused STT ------------------
            psK = ps.tile([P, 3], F32)
            te.matmul(out=psK[0:P, 0:3], lhsT=cm[:], rhs=w3[:],
                      start=True, stop=True)
            junk3 = sb.tile([P, 3], F32)
            t1 = sb.tile([P, 1], F32)          # dup count for this row
            v.scalar_tensor_tensor(out=junk3[:], in0=psK[:], scalar=1.0,
                                   in1=w3[:], op0=A.mult, op1=A.mult,
                                   accum_out=t1[:])
            neglive = sb.tile([P, 1], BF16)    # min(t,1) - valid = -(live)
            v.tensor_scalar(out=neglive[:], in0=t1[:], scalar1=1.0,
                            scalar2=validt[:, 0:1], op0=A.min,
                            op1=A.subtract)

            # ---------------- select + losses ----------------------------
            prod18 = sb.tile([P, 18], F32)
            v.tensor_tensor(out=prod18[:, 0:18].rearrange("p (c a) -> p c a", c=6),
                            in0=cand[:, 0:18].rearrange("p (c a) -> p c a", c=6),
                            in1=w3[:, 0:3].rearrange("p (o a) -> p o a", o=1)
                                .to_broadcast((P, 6, NA)),
                            op=A.mult)
            with nc.allow_low_precision("bf16 loss columns, ~0.4% << 2e-2 gate"):
                v.tensor_reduce(out=loss[:, 0:6],
                                in_=prod18[:, 0:18]
                                    .rearrange("p (c a) -> p c a", c=6),
                                axis=AX.X, op=A.add)

            # ---------------- final row reduction + output ---------------
            psL = ps.tile([P, 8], F32)
            te.matmul(out=psL[0:1, 0:8], lhsT=neglive[:, 0:1],
                      rhs=loss[:, 0:8], start=True, stop=True)
            outs = sb.tile([P, 8], F32)
            v.tensor_copy(out=outs[0:1, :], in_=psL[0:1, :])
            sy.dma_start(out=out_d[:], in_=outs[0:1, 0:8])

    orig = _patch_act_tables()
    try:
        nc.compile()
    finally:
        _unpatch_act_tables(orig)
    return nc


_CACHE = {}


def _get_nc():
    if "nc" not in _CACHE:
        _CACHE["nc"] = _build()
    return _CACHE["nc"]


def _make_cy(y_true_shard, anchors):
    """Per-core y_true + per-partition index constants: [P, CYW]."""
    cy = np.zeros((P, CYW), np.float32)
    b_local = np.repeat(np.arange(BPC), NGT).astype(np.float32)
    cy[:, 0:4] = [0.5, 0.5, 0.25, 0.25]     # pad rows: benign box
    cy[:NGTC, 0:5] = y_true_shard.reshape(NGTC, 5)
    cy[:NGTC, 5] = b_local * PLANE * CHP
    cy[:, 8:14] = anchors.T.reshape(1, 6)    # c-major anchors
    cy[:, 16:31] = np.repeat([1.0, 1.0, 0.0, 0.0, 1.0], NA)[None, :]
    return np.ascontiguousarray(cy)


def make_in_maps(x, y_true, anchors):
    x = np.asarray(x, np.float32)
    y_true = np.ascontiguousarray(y_true, np.float32)
    anchors = np.asarray(anchors, np.float32)
    # channel-last, anchor-interleaved (c-major), padded to 256 channels:
    # [b, gj, gi, c*3+a] so one cell's channels are one aligned 1KB run
    # and a class's 3 anchor logits are contiguous (layout permutation
    # only, applied while sharding).
    xt = np.zeros((BS, GS, GS, CHP), np.float32)
    xt[..., :NCH] = (x.reshape(BS, NA, CH, GS, GS)
                     .transpose(0, 3, 4, 2, 1).reshape(BS, GS, GS, NCH))
    in_maps = []
    for c in range(NCORES):
        in_maps.append({
            "xf": xt[c * BPC:(c + 1) * BPC].reshape(XF2, 1),
            "cy": _make_cy(y_true[c * BPC:(c + 1) * BPC], anchors),
        })
    return in_maps


def combine_outputs(results):
    cols = np.stack([np.asarray(r["out"], np.float64)[0] for r in results])
    tot = -cols.sum(axis=0)      # device sums are weighted by -(live)
    n_live = tot[6]
    out = np.empty(6, np.float64)
    out[0:4] = tot[0:4]
    out[4] = tot[5] + (BS * NA * PLANE - n_live) * np.log(np.float64(NCLS))
    out[5] = tot[4] * 25.0
    return out.astype(np.float32)


def run(x, y_true, anchors, trace=False, **kwargs):
    nc = _get_nc()
    res = run_bass_kernel_spmd(nc, make_in_maps(x, y_true, anchors),
                               list(range(NCORES)), trace=trace, **kwargs)
    return combine_outputs(res.results), res


def kernel(x, y_true, anchors):
    out, _ = run(x, y_true, anchors)
    return out
